# revision 8
# baseline (speedup 1.0000x reference)
"""Trainium2 kernel for nn_DAD_MA_35330400976941 (pairwise-MSE gram loss).

reference math (fm_s is ignored — the original source overwrites G_s with the
teacher matrix and compares against zeros):
    x   = fm_t.reshape(1024, 16384)
    g   = x @ x.T                       # [N, N]
    sq  = diag(g)
    G   = (sq[:,None] + sq[None,:] - 2*g) / D
    out = mean(G**2)                    # scalar f32

Distribution (8 cores, SPMD): circulant row-block scheme over the symmetric
gram. Core c owns row-block c (128 rows) and computes gram[rows_c, cols] for
640 wrapped columns starting at its own diagonal block (block offsets
j=0..4). Offsets j=1..3 are counted twice (their transposes are never
computed), j=0 (diagonal) once, j=4 once (its transpose is computed by the
opposite core). Weights are uniform across cores, so they compile into the
program and the per-core partial sums just add up on the host.

Per-core device work: load its [16384, 640] bf16 operand panel into SBUF
(~20 MB, fully resident), run 128 accumulating matmuls into two PSUM tiles
(N=512 + N=128), then a short DVE epilogue computing
w * sum((sq_i + sq_j - 2*g)^2) per column-chunk. Host sums 8x[128,3] f32
partials and scales by 1/(N^2 * D^2).
"""

import sys

import numpy as np
import ml_dtypes

if "/opt/trn_rl_repo" not in sys.path:
    sys.path.insert(0, "/opt/trn_rl_repo")

N = 1024
D = 16384
NC = 8
BLK = 128            # rows per core
NBLK = 5             # circulant column blocks per core
NCOLS = NBLK * BLK   # 640
KT = 128             # contraction tiles of 128
KG = 4               # k-tiles per DMA group
NKG = KT // KG       # 32 DMA groups
N0 = 512             # first psum tile width
N1 = NCOLS - N0      # second psum tile width

_CACHE = {}


def _build_nc(repeats: int = 1):
    import concourse.bacc as bacc
    import concourse.mybir as mybir
    import concourse.tile as tile

    nc = bacc.Bacc("TRN2", target_bir_lowering=False, debug=False, num_devices=NC)

    xr = nc.dram_tensor("xr", [D, NCOLS], mybir.dt.bfloat16, kind="ExternalInput")
    sqj = nc.dram_tensor("sqj", [BLK, NCOLS], mybir.dt.float32, kind="ExternalInput")
    sqm = nc.dram_tensor("sqm", [BLK, 1], mybir.dt.float32, kind="ExternalInput")
    acc = nc.dram_tensor("acc", [BLK, 3], mybir.dt.float32, kind="ExternalOutput")

    f32 = mybir.dt.float32
    bf16 = mybir.dt.bfloat16
    op = mybir.AluOpType

    with tile.TileContext(nc) as tc:
        with (
            tc.tile_pool(name="xs", bufs=NKG) as xs,
            tc.tile_pool(name="small", bufs=1) as small,
            tc.tile_pool(name="ep", bufs=1) as ep,
            tc.tile_pool(name="ps", bufs=1, space="PSUM") as ps,
        ):
            sqj_sb = small.tile([BLK, NCOLS], f32, tag="sqj")
            nc.sync.dma_start(out=sqj_sb, in_=sqj.ap())
            sqm_sb = small.tile([BLK, 1], f32, tag="sqm")
            nc.sync.dma_start(out=sqm_sb, in_=sqm.ap())

            # [D, NCOLS] -> [NKG, 128, KG, NCOLS]
            xr_r = xr.rearrange("(g k p) n -> g p k n", k=KG, p=BLK)

            acc_sb = ep.tile([BLK, 3], f32, tag="acc")

            for rep in range(repeats):
                xtiles = []
                for g in range(NKG):
                    t = xs.tile([BLK, KG, NCOLS], bf16, tag="xpanel")
                    nc.sync.dma_start(out=t, in_=xr_r[g])
                    xtiles.append(t)

                psum0 = ps.tile([BLK, N0], f32, tag="p0")
                psum1 = ps.tile([BLK, N1], f32, tag="p1")

                for g in range(NKG):
                    t = xtiles[g]
                    for j in range(KG):
                        kt = g * KG + j
                        first = kt == 0
                        last = kt == KT - 1
                        nc.tensor.matmul(
                            psum0,
                            lhsT=t[:, j, 0:BLK],
                            rhs=t[:, j, 0:N0],
                            start=first,
                            stop=last,
                        )
                        nc.tensor.matmul(
                            psum1,
                            lhsT=t[:, j, 0:BLK],
                            rhs=t[:, j, N0:NCOLS],
                            start=first,
                            stop=last,
                        )

                # epilogue: per chunk ci, acc[:, ci] = sum_j (sq_i+sq_j-2g)^2
                # (host applies the {1,2,1} circulant weights)
                chunks = [(0, BLK), (BLK, N0), (N0, NCOLS)]
                for ci, (c0, c1) in enumerate(chunks):
                    width = c1 - c0
                    if c1 <= N0:
                        src = psum0[:, c0:c1]
                    else:
                        src = psum1[:, c0 - N0 : c1 - N0]
                    u = ep.tile([BLK, width], f32, tag=f"u{ci}")
                    tt = ep.tile([BLK, width], f32, tag=f"t{ci}")
                    sq2 = ep.tile([BLK, width], f32, tag=f"s{ci}")
                    # u = -2*g + sq_j
                    nc.vector.scalar_tensor_tensor(
                        out=u,
                        in0=src,
                        scalar=-2.0,
                        in1=sqj_sb[:, c0:c1],
                        op0=op.mult,
                        op1=op.add,
                    )
                    # tt = u + sq_i   (per-partition scalar)
                    nc.vector.tensor_scalar(
                        out=tt,
                        in0=u,
                        scalar1=sqm_sb,
                        scalar2=None,
                        op0=op.add,
                    )
                    # sq2 = tt*tt ; acc[:, ci] = sum_j sq2
                    nc.vector.tensor_tensor(out=sq2, in0=tt, in1=tt, op=op.mult)
                    nc.vector.tensor_reduce(
                        out=acc_sb[:, ci : ci + 1],
                        in_=sq2,
                        axis=mybir.AxisListType.X,
                        op=op.add,
                    )

            nc.sync.dma_start(out=acc.ap(), in_=acc_sb)

    nc.finalize()
    return nc


def _get_nc(repeats: int = 1):
    key = ("nc", repeats)
    if key not in _CACHE:
        _CACHE[key] = _build_nc(repeats)
    return _CACHE[key]


def _prepare_in_maps(fm_t: np.ndarray):
    x = np.ascontiguousarray(np.asarray(fm_t).reshape(N, D))
    xb = x.astype(ml_dtypes.bfloat16)
    # sq from the bf16-rounded values so the diagonal cancels against the
    # bf16 gram; accumulate in f64 for a stable f32 result.
    sq = (xb.astype(np.float64) ** 2).sum(axis=1).astype(np.float32)
    xT = np.ascontiguousarray(xb.T)  # [D, N] bf16

    in_maps = []
    for c in range(NC):
        cols = np.arange(c * BLK, c * BLK + NCOLS) % N
        xr_c = np.ascontiguousarray(xT[:, cols])
        sqj_c = np.ascontiguousarray(
            np.broadcast_to(sq[cols][None, :], (BLK, NCOLS))
        ).astype(np.float32)
        sqm_c = sq[c * BLK : (c + 1) * BLK].reshape(BLK, 1).copy()
        in_maps.append({"xr": xr_c, "sqj": sqj_c, "sqm": sqm_c})
    return in_maps


def run(fm_t: np.ndarray, trace: bool = False, repeats: int = 1, in_maps=None):
    """Returns (loss_f32, BassKernelResults)."""
    from concourse.bass_utils import run_bass_kernel_spmd

    nc = _get_nc(repeats)
    if in_maps is None:
        in_maps = _prepare_in_maps(fm_t)
    res = run_bass_kernel_spmd(nc, in_maps, list(range(NC)), trace=trace)
    w = np.array([1.0, 2.0, 1.0], dtype=np.float64)
    tot = 0.0
    for r in res.results:
        tot += float((r["acc"].astype(np.float64) * w[None, :]).sum())
    loss = tot / (float(N) ** 2 * float(D) ** 2)
    return np.float32(loss), res


def kernel(fm_s: np.ndarray, fm_t: np.ndarray) -> np.ndarray:
    loss, _ = run(fm_t, trace=False)
    return np.asarray(loss, dtype=np.float32)


# revision 17
# speedup vs baseline: 479.1730x; 479.1730x over previous
"""Trainium2 kernel for nn_DAD_MA_35330400976941 (pairwise-MSE gram loss).

reference math (fm_s is ignored — the original source overwrites G_s with the
teacher matrix and compares against zeros):
    x   = fm_t.reshape(1024, 16384)
    g   = x @ x.T                       # [N, N]
    sq  = diag(g)
    G   = (sq[:,None] + sq[None,:] - 2*g) / D
    out = mean(G**2)                    # scalar f32

Distribution (8 cores, SPMD): circulant row-block scheme over the symmetric
gram. Core c owns row-block c (128 rows) and computes gram[rows_c, cols] for
640 wrapped columns starting at its own diagonal block (block offsets
j=0..4). Offsets j=1..3 are counted twice (their transposes are never
computed), j=0 (diagonal) once, j=4 once (its transpose is computed by the
opposite core). Weights are uniform across cores, so the per-core partial
sums just add up on the host with fixed weights.

The sq terms are folded into the GEMM: the host appends one extra k-tile
with 4 live rows (ones paired against -sq/2 split into bf16 hi+lo for
precision, and the mirror) so PSUM accumulates t' = g - (sq_i+sq_j)/2
= -t/2 directly; the diagonal cancels exactly in fp32 PSUM. The epilogue is
then just square+reduce per column chunk on DVE, and the host multiplies the
partials by 4.

Per-core device work: load its [16384+256, 640] bf16 operand panel into SBUF
(~20 MB, fully resident), 130 accumulating matmuls into two PSUM tiles
(N=512 + N=128), square+reduce, DMA out [128, 3] f32 partials.
"""

import sys

import numpy as np
import ml_dtypes

if "/opt/trn_rl_repo" not in sys.path:
    sys.path.insert(0, "/opt/trn_rl_repo")

N = 1024
D = 16384
NC = 8
BLK = 128            # rows per core
NBLK = 5             # circulant column blocks per core
NCOLS = NBLK * BLK   # 640
KT = 128             # contraction tiles of 128
KG = 4               # k-tiles per DMA group
NKG = KT // KG       # 32 DMA groups
DX = D + 2 * BLK     # input rows incl. the L/R extra tiles
N0 = 512             # first psum tile width
N1 = NCOLS - N0      # second psum tile width

_CACHE = {}


def _build_nc(repeats: int = 1, kg: int = KG, in_dt: str = "bfloat16"):
    import concourse.bacc as bacc
    import concourse.mybir as mybir
    import concourse.tile as tile

    nc = bacc.Bacc("TRN2", target_bir_lowering=False, debug=False, num_devices=NC)

    xdt = getattr(mybir.dt, in_dt)
    f32 = mybir.dt.float32
    op = mybir.AluOpType
    nkg = KT // kg

    xr = nc.dram_tensor("xr", [DX, NCOLS], xdt, kind="ExternalInput")
    acc = nc.dram_tensor("acc", [BLK, 3], f32, kind="ExternalOutput")

    with tile.TileContext(nc) as tc:
        with (
            tc.tile_pool(name="xs", bufs=nkg) as xs,
            tc.tile_pool(name="xe", bufs=1) as xe,
            tc.tile_pool(name="ep", bufs=1) as ep,
            tc.tile_pool(name="ps", bufs=1, space="PSUM") as ps,
        ):
            # [D, NCOLS] part -> [nkg, 128, kg, NCOLS]
            xr_r = xr[0:D, :].rearrange("(g k p) n -> g p k n", k=kg, p=BLK)
            # extra L/R tiles, AP ordered to match the SBUF tile [p, e, n]
            xr_e = xr[D:DX, :].rearrange("(e p) n -> p e n", p=BLK)

            acc_sb = ep.tile([BLK, 3], f32, tag="acc")

            for _rep in range(repeats):
                xtiles = []
                for g in range(nkg):
                    t = xs.tile([BLK, kg, NCOLS], xdt, tag="xpanel")
                    nc.sync.dma_start(out=t, in_=xr_r[g])
                    xtiles.append(t)
                ex = xe.tile([BLK, 2, NCOLS], xdt, tag="xextra")
                nc.sync.dma_start(out=ex, in_=xr_e)

                psum0 = ps.tile([BLK, N0], f32, tag="p0")
                psum1 = ps.tile([BLK, N1], f32, tag="p1")

                for g in range(nkg):
                    t = xtiles[g]
                    for j in range(kg):
                        first = g == 0 and j == 0
                        nc.tensor.matmul(
                            psum0,
                            lhsT=t[:, j, 0:BLK],
                            rhs=t[:, j, 0:N0],
                            start=first,
                            stop=False,
                        )
                        nc.tensor.matmul(
                            psum1,
                            lhsT=t[:, j, 0:BLK],
                            rhs=t[:, j, N0:NCOLS],
                            start=first,
                            stop=False,
                        )
                # fold in -(sq_i+sq_j)/2 via the L/R extra tile pair
                nc.tensor.matmul(
                    psum0,
                    lhsT=ex[:, 0, 0:BLK],
                    rhs=ex[:, 1, 0:N0],
                    start=False,
                    stop=True,
                )
                nc.tensor.matmul(
                    psum1,
                    lhsT=ex[:, 0, 0:BLK],
                    rhs=ex[:, 1, N0:NCOLS],
                    start=False,
                    stop=True,
                )

                # epilogue: acc[:, ci] = sum_j t'^2 per chunk (host scales by 4w)
                chunks = [(0, BLK), (BLK, N0), (N0, NCOLS)]
                for ci, (c0, c1) in enumerate(chunks):
                    width = c1 - c0
                    if c1 <= N0:
                        src = psum0[:, c0:c1]
                    else:
                        src = psum1[:, c0 - N0 : c1 - N0]
                    u = ep.tile([BLK, width], f32, tag=f"u{ci}")
                    sq2 = ep.tile([BLK, width], f32, tag=f"s{ci}")
                    nc.vector.tensor_copy(out=u, in_=src)
                    nc.vector.tensor_tensor(out=sq2, in0=u, in1=u, op=op.mult)
                    nc.vector.tensor_reduce(
                        out=acc_sb[:, ci : ci + 1],
                        in_=sq2,
                        axis=mybir.AxisListType.X,
                        op=op.add,
                    )

            nc.sync.dma_start(out=acc.ap(), in_=acc_sb)

    nc.finalize()
    return nc


def _get_nc(repeats: int = 1):
    key = ("nc", repeats)
    if key not in _CACHE:
        _CACHE[key] = _build_nc(repeats)
    return _CACHE[key]


def _prepare_in_maps(fm_t: np.ndarray):
    bf16 = ml_dtypes.bfloat16
    x = np.ascontiguousarray(np.asarray(fm_t).reshape(N, D))
    xb = x.astype(bf16)
    # sq of the bf16-rounded rows, accumulated in f64; split sq/2 into
    # bf16 hi+lo so the GEMM-folded sq matches fp32 precision.
    sq = (xb.astype(np.float64) ** 2).sum(axis=1)
    half = (sq / 2.0).astype(np.float32)
    hi = half.astype(bf16)
    lo = (half - hi.astype(np.float32)).astype(bf16)
    xT = np.ascontiguousarray(xb.T)  # [D, N] bf16

    in_maps = []
    for c in range(NC):
        cols = np.arange(c * BLK, c * BLK + NCOLS) % N
        xr_c = np.empty((DX, NCOLS), dtype=bf16)
        xr_c[0:D] = xT[:, cols]
        # L tile rows (lhsT side, k-tile index 128)
        L = np.zeros((BLK, NCOLS), dtype=bf16)
        L[0, :] = bf16(1.0)
        L[1, :] = bf16(1.0)
        L[2, :] = hi[cols]
        L[3, :] = lo[cols]
        # R tile rows (rhs side, k-tile index 129)
        R = np.zeros((BLK, NCOLS), dtype=bf16)
        R[0, :] = -hi[cols]
        R[1, :] = -lo[cols]
        R[2, :] = bf16(-1.0)
        R[3, :] = bf16(-1.0)
        xr_c[D : D + BLK] = L
        xr_c[D + BLK : DX] = R
        in_maps.append({"xr": xr_c})
    return in_maps


def run(fm_t: np.ndarray, trace: bool = False, repeats: int = 1, in_maps=None):
    """Returns (loss_f32, BassKernelResults)."""
    from concourse.bass_utils import run_bass_kernel_spmd

    nc = _get_nc(repeats)
    if in_maps is None:
        in_maps = _prepare_in_maps(fm_t)
    res = run_bass_kernel_spmd(nc, in_maps, list(range(NC)), trace=trace)
    # psum held t' = -t/2, so t^2 = 4*t'^2; circulant chunk weights {1,2,1}
    w = np.array([4.0, 8.0, 4.0], dtype=np.float64)
    tot = 0.0
    for r in res.results:
        tot += float((r["acc"].astype(np.float64) * w[None, :]).sum())
    loss = tot / (float(N) ** 2 * float(D) ** 2)
    return np.float32(loss), res


def kernel(fm_s: np.ndarray, fm_t: np.ndarray) -> np.ndarray:
    loss, _ = run(fm_t, trace=False)
    return np.asarray(loss, dtype=np.float32)


# revision 18
# speedup vs baseline: 670.2442x; 1.3988x over previous
"""Trainium2 kernel for nn_DAD_MA_35330400976941 (pairwise-MSE gram loss).

reference math (fm_s is ignored — the original source overwrites G_s with the
teacher matrix and compares against zeros):
    x   = fm_t.reshape(1024, 16384)
    g   = x @ x.T                       # [N, N]
    sq  = diag(g)
    G   = (sq[:,None] + sq[None,:] - 2*g) / D
    out = mean(G**2)                    # scalar f32

Algorithm: expand the sum,
    sum_ij t^2 = 2*N*sum(sq^2) + 2*(sum sq)^2 - 8*sum_i sq_i*(x_i . s)
                 + 4*sum_ij g_ij^2,          t = sq_i + sq_j - 2 g_ij
with s = sum_j x_j. All O(N*D) terms are computed on the host in float64
from the exact f32 input. Only the O(N^2*D) term Qg = sum_ij g_ij^2 runs on
the device. Qg contributes only ~0.1% of the total, so the device gram can
use fp8e4m3 inputs (fp32 PSUM accumulation): its ~1e-3 relative error on Qg
turns into ~3e-6 on the loss while halving the DMA bytes vs bf16.

Distribution (8 cores, SPMD): circulant row-block scheme over the symmetric
gram. Core c owns row-block c (128 rows) and computes g[rows_c, cols] for
640 wrapped columns starting at its own diagonal block (block offsets
j=0..4). Offsets j=1..3 are counted twice (their transposes are never
computed), j=0 (diagonal) once, j=4 once (its transpose is computed by the
opposite core). Weights are core-uniform, applied on the host: chunk sums
arrive as [128, 3] f32 per core for chunks {j=0}, {j=1..3}, {j=4} with
weights {1, 2, 1}.

Per-core device work: load its [16384, 640] fp8 panel into SBUF (~10.5 MB,
fully resident, 32 DMA groups), 256 accumulating matmuls into two PSUM
tiles (N=512 + N=128), square+reduce per chunk, DMA out [128, 3] f32.
"""

import sys

import numpy as np
import ml_dtypes

if "/opt/trn_rl_repo" not in sys.path:
    sys.path.insert(0, "/opt/trn_rl_repo")

N = 1024
D = 16384
NC = 8
BLK = 128            # rows per core
NBLK = 5             # circulant column blocks per core
NCOLS = NBLK * BLK   # 640
KT = 128             # contraction tiles of 128
KG = 4               # k-tiles per DMA group
NKG = KT // KG       # 32 DMA groups
N0 = 512             # first psum tile width
N1 = NCOLS - N0      # second psum tile width

_CACHE = {}


def _build_nc(repeats: int = 1, kg: int = KG, in_dt: str = "float8e4"):
    import concourse.bacc as bacc
    import concourse.mybir as mybir
    import concourse.tile as tile

    nc = bacc.Bacc("TRN2", target_bir_lowering=False, debug=False, num_devices=NC)

    xdt = getattr(mybir.dt, in_dt)
    f32 = mybir.dt.float32
    op = mybir.AluOpType
    nkg = KT // kg

    xr = nc.dram_tensor("xr", [D, NCOLS], xdt, kind="ExternalInput")
    acc = nc.dram_tensor("acc", [BLK, 3], f32, kind="ExternalOutput")

    with tile.TileContext(nc) as tc:
        with (
            tc.tile_pool(name="xs", bufs=nkg) as xs,
            tc.tile_pool(name="ep", bufs=1) as ep,
            tc.tile_pool(name="ps", bufs=1, space="PSUM") as ps,
        ):
            # [D, NCOLS] -> [nkg, 128, kg, NCOLS]
            xr_r = xr.rearrange("(g k p) n -> g p k n", k=kg, p=BLK)

            acc_sb = ep.tile([BLK, 3], f32, tag="acc")

            for _rep in range(repeats):
                xtiles = []
                for g in range(nkg):
                    t = xs.tile([BLK, kg, NCOLS], xdt, tag="xpanel")
                    nc.sync.dma_start(out=t, in_=xr_r[g])
                    xtiles.append(t)

                psum0 = ps.tile([BLK, N0], f32, tag="p0")
                psum1 = ps.tile([BLK, N1], f32, tag="p1")

                for g in range(nkg):
                    t = xtiles[g]
                    for j in range(kg):
                        kt = g * kg + j
                        first = kt == 0
                        last = kt == KT - 1
                        nc.tensor.matmul(
                            psum0,
                            lhsT=t[:, j, 0:BLK],
                            rhs=t[:, j, 0:N0],
                            start=first,
                            stop=last,
                        )
                        nc.tensor.matmul(
                            psum1,
                            lhsT=t[:, j, 0:BLK],
                            rhs=t[:, j, N0:NCOLS],
                            start=first,
                            stop=last,
                        )

                # epilogue: acc[:, ci] = sum_j g^2 per chunk
                chunks = [(0, BLK), (BLK, N0), (N0, NCOLS)]
                for ci, (c0, c1) in enumerate(chunks):
                    width = c1 - c0
                    if c1 <= N0:
                        src = psum0[:, c0:c1]
                    else:
                        src = psum1[:, c0 - N0 : c1 - N0]
                    u = ep.tile([BLK, width], f32, tag=f"u{ci}")
                    sq2 = ep.tile([BLK, width], f32, tag=f"s{ci}")
                    nc.vector.tensor_copy(out=u, in_=src)
                    nc.vector.tensor_tensor(out=sq2, in0=u, in1=u, op=op.mult)
                    nc.vector.tensor_reduce(
                        out=acc_sb[:, ci : ci + 1],
                        in_=sq2,
                        axis=mybir.AxisListType.X,
                        op=op.add,
                    )

            nc.sync.dma_start(out=acc.ap(), in_=acc_sb)

    nc.finalize()
    return nc


def _get_nc(repeats: int = 1):
    key = ("nc", repeats)
    if key not in _CACHE:
        _CACHE[key] = _build_nc(repeats)
    return _CACHE[key]


def _prepare_in_maps(fm_t: np.ndarray):
    fp8 = ml_dtypes.float8_e4m3
    x = np.ascontiguousarray(np.asarray(fm_t).reshape(N, D))
    xT = np.ascontiguousarray(x.astype(fp8).T)  # [D, N] fp8

    in_maps = []
    for c in range(NC):
        cols = np.arange(c * BLK, c * BLK + NCOLS) % N
        in_maps.append({"xr": np.ascontiguousarray(xT[:, cols])})
    return in_maps


def _host_terms(fm_t: np.ndarray):
    """f64 O(N*D) side terms of the expansion from the exact f32 input."""
    x = np.asarray(fm_t).reshape(N, D).astype(np.float64)
    sq = (x * x).sum(axis=1)          # [N]
    s = x.sum(axis=0)                 # [D]
    r = x @ s                         # [N]
    A = float((sq * sq).sum())
    S1 = float(sq.sum())
    B = float((sq * r).sum())
    return A, S1, B


def run(fm_t: np.ndarray, trace: bool = False, repeats: int = 1, in_maps=None):
    """Returns (loss_f32, BassKernelResults)."""
    from concourse.bass_utils import run_bass_kernel_spmd

    nc = _get_nc(repeats)
    if in_maps is None:
        in_maps = _prepare_in_maps(fm_t)
    res = run_bass_kernel_spmd(nc, in_maps, list(range(NC)), trace=trace)

    # Qg = weighted sum of g^2 over the circulant chunks
    w = np.array([1.0, 2.0, 1.0], dtype=np.float64)
    qg = 0.0
    for r_ in res.results:
        qg += float((r_["acc"].astype(np.float64) * w[None, :]).sum())

    A, S1, B = _host_terms(fm_t)
    tot = 2.0 * N * A + 2.0 * S1 * S1 - 8.0 * B + 4.0 * qg
    loss = tot / (float(N) ** 2 * float(D) ** 2)
    return np.float32(loss), res


def kernel(fm_s: np.ndarray, fm_t: np.ndarray) -> np.ndarray:
    loss, _ = run(fm_t, trace=False)
    return np.asarray(loss, dtype=np.float32)


# revision 19
# speedup vs baseline: 885.8890x; 1.3217x over previous
"""Trainium2 kernel for nn_DAD_MA_35330400976941 (pairwise-MSE gram loss).

reference math (fm_s is ignored — the original source overwrites G_s with the
teacher matrix and compares against zeros):
    x   = fm_t.reshape(1024, 16384)
    g   = x @ x.T                       # [N, N]
    sq  = diag(g)
    G   = (sq[:,None] + sq[None,:] - 2*g) / D
    out = mean(G**2)                    # scalar f32

Algorithm: expand the sum,
    sum_ij t^2 = 2*N*sum(sq^2) + 2*(sum sq)^2 - 8*sum_i sq_i*(x_i . s)
                 + 4*sum_ij g_ij^2,          t = sq_i + sq_j - 2 g_ij
with s = sum_j x_j. All O(N*D) terms are computed on the host in float64
from the exact f32 input. Only the O(N^2*D) term Qg = sum_ij g_ij^2 runs on
the device. Qg contributes only ~0.1% of the total, so the device gram can
use fp8e4m3 inputs (fp32 PSUM accumulation): its ~1e-3 relative error on Qg
turns into ~3e-6 on the loss while halving the DMA bytes vs bf16.

Distribution (8 cores, SPMD): circulant row-block scheme over the symmetric
gram. Core c owns row-block c (128 rows) and computes g[rows_c, cols] for
640 wrapped columns starting at its own diagonal block (block offsets
j=0..4). Offsets j=1..3 are counted twice (their transposes are never
computed), j=0 (diagonal) once, j=4 once (its transpose is computed by the
opposite core). Weights are core-uniform, applied on the host: chunk sums
arrive as [128, 3] f32 per core for chunks {j=0}, {j=1..3}, {j=4} with
weights {1, 2, 1}.

Per-core device work: load its [16384, 640] fp8 panel into SBUF (~10.5 MB,
fully resident, 32 DMA groups), 256 accumulating matmuls into two PSUM
tiles (N=512 + N=128), square+reduce per chunk, DMA out [128, 3] f32.
"""

import sys

import numpy as np
import ml_dtypes

if "/opt/trn_rl_repo" not in sys.path:
    sys.path.insert(0, "/opt/trn_rl_repo")

N = 1024
D = 16384
NC = 8
BLK = 128            # rows per core
NBLK = 5             # circulant column blocks per core
NCOLS = NBLK * BLK   # 640
KT = 128             # contraction tiles of 128
KG = 4               # k-tiles per DMA group
NKG = KT // KG       # 32 DMA groups
N0 = 512             # first psum tile width
N1 = NCOLS - N0      # second psum tile width

_CACHE = {}


def _build_nc(repeats: int = 1, kg: int = KG, in_dt: str = "float8e4"):
    import concourse.bacc as bacc
    import concourse.mybir as mybir
    import concourse.tile as tile

    nc = bacc.Bacc("TRN2", target_bir_lowering=False, debug=False, num_devices=NC)

    xdt = getattr(mybir.dt, in_dt)
    f32 = mybir.dt.float32
    op = mybir.AluOpType
    nkg = KT // kg

    xr = nc.dram_tensor("xr", [D, NCOLS], xdt, kind="ExternalInput")
    acc = nc.dram_tensor("acc", [BLK, 3], f32, kind="ExternalOutput")

    with tile.TileContext(nc) as tc:
        with (
            tc.tile_pool(name="xs", bufs=nkg) as xs,
            tc.tile_pool(name="ep", bufs=1) as ep,
            tc.tile_pool(name="ps", bufs=1, space="PSUM") as ps,
        ):
            # [D, NCOLS] -> [nkg, 128, kg, NCOLS]
            xr_r = xr.rearrange("(g k p) n -> g p k n", k=kg, p=BLK)

            acc_sb = ep.tile([BLK, 3], f32, tag="acc")

            for _rep in range(repeats):
                xtiles = []
                for g in range(nkg):
                    t = xs.tile([BLK, kg, NCOLS], xdt, tag="xpanel")
                    nc.sync.dma_start(out=t, in_=xr_r[g])
                    xtiles.append(t)

                psum0 = ps.tile([BLK, N0], f32, tag="p0")
                psum1 = ps.tile([BLK, N1], f32, tag="p1")

                # fp8 DoubleRow: each matmul contracts a pair of k-tiles
                # (128 partitions x 2 fp8 elements per cell)
                for g in range(nkg):
                    t = xtiles[g]
                    for jp in range(kg // 2):
                        ktp = g * (kg // 2) + jp
                        first = ktp == 0
                        last = ktp == KT // 2 - 1
                        nc.tensor.matmul(
                            psum0,
                            lhsT=t[:, 2 * jp : 2 * jp + 2, 0:BLK],
                            rhs=t[:, 2 * jp : 2 * jp + 2, 0:N0],
                            perf_mode=mybir.MatmulPerfMode.DoubleRow,
                            start=first,
                            stop=last,
                        )
                        nc.tensor.matmul(
                            psum1,
                            lhsT=t[:, 2 * jp : 2 * jp + 2, 0:BLK],
                            rhs=t[:, 2 * jp : 2 * jp + 2, N0:NCOLS],
                            perf_mode=mybir.MatmulPerfMode.DoubleRow,
                            start=first,
                            stop=last,
                        )

                # epilogue: acc[:, ci] = sum_j g^2 per chunk, fused on ACT
                # (device-verified: Square is exact, accum_out sums free dim)
                chunks = [(0, BLK), (BLK, N0), (N0, NCOLS)]
                for ci, (c0, c1) in enumerate(chunks):
                    width = c1 - c0
                    if c1 <= N0:
                        src = psum0[:, c0:c1]
                    else:
                        src = psum1[:, c0 - N0 : c1 - N0]
                    scr = ep.tile([BLK, width], f32, tag=f"s{ci}")
                    nc.scalar.activation(
                        out=scr,
                        in_=src,
                        func=mybir.ActivationFunctionType.Square,
                        accum_out=acc_sb[:, ci : ci + 1],
                    )

            nc.sync.dma_start(out=acc.ap(), in_=acc_sb)

    nc.finalize()
    return nc


def _get_nc(repeats: int = 1):
    key = ("nc", repeats)
    if key not in _CACHE:
        _CACHE[key] = _build_nc(repeats)
    return _CACHE[key]


def _prepare_in_maps(fm_t: np.ndarray):
    fp8 = ml_dtypes.float8_e4m3
    x = np.ascontiguousarray(np.asarray(fm_t).reshape(N, D))
    xT = np.ascontiguousarray(x.astype(fp8).T)  # [D, N] fp8

    in_maps = []
    for c in range(NC):
        cols = np.arange(c * BLK, c * BLK + NCOLS) % N
        in_maps.append({"xr": np.ascontiguousarray(xT[:, cols])})
    return in_maps


def _host_terms(fm_t: np.ndarray):
    """f64 O(N*D) side terms of the expansion from the exact f32 input."""
    x = np.asarray(fm_t).reshape(N, D).astype(np.float64)
    sq = (x * x).sum(axis=1)          # [N]
    s = x.sum(axis=0)                 # [D]
    r = x @ s                         # [N]
    A = float((sq * sq).sum())
    S1 = float(sq.sum())
    B = float((sq * r).sum())
    return A, S1, B


def run(fm_t: np.ndarray, trace: bool = False, repeats: int = 1, in_maps=None):
    """Returns (loss_f32, BassKernelResults)."""
    from concourse.bass_utils import run_bass_kernel_spmd

    nc = _get_nc(repeats)
    if in_maps is None:
        in_maps = _prepare_in_maps(fm_t)
    res = run_bass_kernel_spmd(nc, in_maps, list(range(NC)), trace=trace)

    # Qg = weighted sum of g^2 over the circulant chunks
    w = np.array([1.0, 2.0, 1.0], dtype=np.float64)
    qg = 0.0
    for r_ in res.results:
        qg += float((r_["acc"].astype(np.float64) * w[None, :]).sum())

    A, S1, B = _host_terms(fm_t)
    tot = 2.0 * N * A + 2.0 * S1 * S1 - 8.0 * B + 4.0 * qg
    loss = tot / (float(N) ** 2 * float(D) ** 2)
    return np.float32(loss), res


def kernel(fm_s: np.ndarray, fm_t: np.ndarray) -> np.ndarray:
    loss, _ = run(fm_t, trace=False)
    return np.asarray(loss, dtype=np.float32)


# revision 20
# speedup vs baseline: 897.3055x; 1.0129x over previous
"""Trainium2 kernel for nn_DAD_MA_35330400976941 (pairwise-MSE gram loss).

reference math (fm_s is ignored — the original source overwrites G_s with the
teacher matrix and compares against zeros):
    x   = fm_t.reshape(1024, 16384)
    g   = x @ x.T                       # [N, N]
    sq  = diag(g)
    G   = (sq[:,None] + sq[None,:] - 2*g) / D
    out = mean(G**2)                    # scalar f32

Algorithm: expand the sum,
    sum_ij t^2 = 2*N*sum(sq^2) + 2*(sum sq)^2 - 8*sum_i sq_i*(x_i . s)
                 + 4*sum_ij g_ij^2,          t = sq_i + sq_j - 2 g_ij
with s = sum_j x_j. All O(N*D) terms are computed on the host in float64
from the exact f32 input. Only the O(N^2*D) term Qg = sum_ij g_ij^2 runs on
the device. Qg contributes only ~0.1% of the total, so the device gram can
use fp8e4m3 inputs (fp32 PSUM accumulation): its ~1e-3 relative error on Qg
turns into ~3e-6 on the loss while halving the DMA bytes vs bf16.

Distribution (8 cores, SPMD): circulant row-block scheme over the symmetric
gram. Core c owns row-block c (128 rows) and computes g[rows_c, cols] for
640 wrapped columns starting at its own diagonal block (block offsets
j=0..4). Offsets j=1..3 are counted twice (their transposes are never
computed), j=0 (diagonal) once, j=4 once (its transpose is computed by the
opposite core). Weights are core-uniform, applied on the host: chunk sums
arrive as [128, 3] f32 per core for chunks {j=0}, {j=1..3}, {j=4} with
weights {1, 2, 1}.

Per-core device work: load its [16384, 640] fp8 panel into SBUF (~10.5 MB,
fully resident, 32 DMA groups), 256 accumulating matmuls into two PSUM
tiles (N=512 + N=128), square+reduce per chunk, DMA out [128, 3] f32.
"""

import sys

import numpy as np
import ml_dtypes

if "/opt/trn_rl_repo" not in sys.path:
    sys.path.insert(0, "/opt/trn_rl_repo")

N = 1024
D = 16384
NC = 8
BLK = 128            # rows per core
NBLK = 5             # circulant column blocks per core
NCOLS = NBLK * BLK   # 640
KT = 128             # contraction tiles of 128
KG = 4               # k-tiles per DMA group
NKG = KT // KG       # 32 DMA groups
N0 = 512             # first psum tile width
N1 = NCOLS - N0      # second psum tile width

_CACHE = {}


def _build_nc(repeats: int = 1, kg: int = KG, in_dt: str = "float8e4"):
    import concourse.bacc as bacc
    import concourse.mybir as mybir
    import concourse.tile as tile

    nc = bacc.Bacc("TRN2", target_bir_lowering=False, debug=False, num_devices=NC)

    xdt = getattr(mybir.dt, in_dt)
    f32 = mybir.dt.float32
    op = mybir.AluOpType
    nkg = KT // kg

    xr = nc.dram_tensor("xr", [D, NCOLS], xdt, kind="ExternalInput")
    acc = nc.dram_tensor("acc", [BLK, 3], f32, kind="ExternalOutput")

    # tapered DMA plan: big groups, then two 2-k-tile tail groups so the
    # final matmuls wait on a smaller last transfer
    tail_sizes = [2, 2]
    nmain = (KT - sum(tail_sizes)) // kg
    sizes = [kg] * nmain + tail_sizes

    with tile.TileContext(nc) as tc:
        with (
            tc.tile_pool(name="xsA", bufs=nmain) as xsA,
            tc.tile_pool(name="xsB", bufs=len(tail_sizes)) as xsB,
            tc.tile_pool(name="ep", bufs=1) as ep,
            tc.tile_pool(name="ps", bufs=1, space="PSUM") as ps,
        ):
            acc_sb = ep.tile([BLK, 3], f32, tag="acc")

            for _rep in range(repeats):
                xtiles = []
                kt0 = 0
                for i, sz in enumerate(sizes):
                    pool = xsA if i < nmain else xsB
                    t = pool.tile(
                        [BLK, sz, NCOLS], xdt,
                        tag="xpM" if i < nmain else f"xpT{i - nmain}",
                    )
                    nc.sync.dma_start(
                        out=t,
                        in_=xr[kt0 * BLK : (kt0 + sz) * BLK, :].rearrange(
                            "(k p) n -> p k n", p=BLK
                        ),
                    )
                    xtiles.append((sz, t))
                    kt0 += sz

                psum0 = ps.tile([BLK, N0], f32, tag="p0")
                psum1 = ps.tile([BLK, N1], f32, tag="p1")

                # fp8 DoubleRow: each matmul contracts a pair of k-tiles
                # (128 partitions x 2 fp8 elements per cell); psum1 first so
                # its chunk-2 epilogue can start ahead of psum0's last matmul
                npairs = KT // 2
                pi = 0
                for sz, t in xtiles:
                    for jp in range(sz // 2):
                        first = pi == 0
                        last = pi == npairs - 1
                        nc.tensor.matmul(
                            psum1,
                            lhsT=t[:, 2 * jp : 2 * jp + 2, 0:BLK],
                            rhs=t[:, 2 * jp : 2 * jp + 2, N0:NCOLS],
                            perf_mode=mybir.MatmulPerfMode.DoubleRow,
                            start=first,
                            stop=last,
                        )
                        nc.tensor.matmul(
                            psum0,
                            lhsT=t[:, 2 * jp : 2 * jp + 2, 0:BLK],
                            rhs=t[:, 2 * jp : 2 * jp + 2, 0:N0],
                            perf_mode=mybir.MatmulPerfMode.DoubleRow,
                            start=first,
                            stop=last,
                        )
                        pi += 1

                # epilogue: acc[:, ci] = sum_j g^2 per chunk. Chunks 2 and 1
                # fused on ACT (device-verified: Square is exact, accum_out
                # sums the free dim); chunk 0 on DVE in parallel.
                cdef = {0: (0, BLK), 1: (BLK, N0), 2: (N0, NCOLS)}
                for ci in (2, 0, 1):
                    c0, c1 = cdef[ci]
                    width = c1 - c0
                    if c1 <= N0:
                        src = psum0[:, c0:c1]
                    else:
                        src = psum1[:, c0 - N0 : c1 - N0]
                    if ci == 0:
                        u = ep.tile([BLK, width], f32, tag=f"u{ci}")
                        nc.vector.tensor_copy(out=u, in_=src)
                        sq2 = ep.tile([BLK, width], f32, tag=f"q{ci}")
                        nc.vector.tensor_tensor(out=sq2, in0=u, in1=u, op=op.mult)
                        nc.vector.tensor_reduce(
                            out=acc_sb[:, ci : ci + 1],
                            in_=sq2,
                            axis=mybir.AxisListType.X,
                            op=op.add,
                        )
                    else:
                        scr = ep.tile([BLK, width], f32, tag=f"s{ci}")
                        nc.scalar.activation(
                            out=scr,
                            in_=src,
                            func=mybir.ActivationFunctionType.Square,
                            accum_out=acc_sb[:, ci : ci + 1],
                        )

            nc.sync.dma_start(out=acc.ap(), in_=acc_sb)

    nc.finalize()
    return nc


def _get_nc(repeats: int = 1):
    key = ("nc", repeats)
    if key not in _CACHE:
        _CACHE[key] = _build_nc(repeats)
    return _CACHE[key]


def _prepare_in_maps(fm_t: np.ndarray):
    fp8 = ml_dtypes.float8_e4m3
    x = np.ascontiguousarray(np.asarray(fm_t).reshape(N, D))
    xT = np.ascontiguousarray(x.astype(fp8).T)  # [D, N] fp8

    in_maps = []
    for c in range(NC):
        cols = np.arange(c * BLK, c * BLK + NCOLS) % N
        in_maps.append({"xr": np.ascontiguousarray(xT[:, cols])})
    return in_maps


def _host_terms(fm_t: np.ndarray):
    """f64 O(N*D) side terms of the expansion from the exact f32 input."""
    x = np.asarray(fm_t).reshape(N, D).astype(np.float64)
    sq = (x * x).sum(axis=1)          # [N]
    s = x.sum(axis=0)                 # [D]
    r = x @ s                         # [N]
    A = float((sq * sq).sum())
    S1 = float(sq.sum())
    B = float((sq * r).sum())
    return A, S1, B


def run(fm_t: np.ndarray, trace: bool = False, repeats: int = 1, in_maps=None):
    """Returns (loss_f32, BassKernelResults)."""
    from concourse.bass_utils import run_bass_kernel_spmd

    nc = _get_nc(repeats)
    if in_maps is None:
        in_maps = _prepare_in_maps(fm_t)
    res = run_bass_kernel_spmd(nc, in_maps, list(range(NC)), trace=trace)

    # Qg = weighted sum of g^2 over the circulant chunks
    w = np.array([1.0, 2.0, 1.0], dtype=np.float64)
    qg = 0.0
    for r_ in res.results:
        qg += float((r_["acc"].astype(np.float64) * w[None, :]).sum())

    A, S1, B = _host_terms(fm_t)
    tot = 2.0 * N * A + 2.0 * S1 * S1 - 8.0 * B + 4.0 * qg
    loss = tot / (float(N) ** 2 * float(D) ** 2)
    return np.float32(loss), res


def kernel(fm_s: np.ndarray, fm_t: np.ndarray) -> np.ndarray:
    loss, _ = run(fm_t, trace=False)
    return np.asarray(loss, dtype=np.float32)


# revision 22
# speedup vs baseline: 1017.0245x; 1.1334x over previous
"""Trainium2 kernel for nn_DAD_MA_35330400976941 (pairwise-MSE gram loss).

reference math (fm_s is ignored — the original source overwrites G_s with the
teacher matrix and compares against zeros):
    x   = fm_t.reshape(1024, 16384)
    g   = x @ x.T
    sq  = diag(g)
    out = mean(((sq[:,None] + sq[None,:] - 2*g) / D)**2)

Algorithm: expand the sum,
    sum_ij t^2 = 2*N*sum(sq^2) + 2*(sum sq)^2 - 8*sum_i sq_i*(x_i . s)
                 + 4*Qg,      Qg = sum_ij g_ij^2
with s = sum_j x_j. All O(N*D) terms are computed on the host in float64
from the exact f32 input; only Qg (0.1% of the total) runs on the device,
so the device gram uses fp8e4m3 inputs (fp32 PSUM accumulation) — ~1e-3
error on Qg is ~3e-6 on the loss.

Distribution (8 cores, SPMD): Fano-plane covering design over 16 groups of
64 gram rows. Each 64-row group is assigned a set of covering cores (a Fano
line over cores 0-6, or {7} u L0); any two lines intersect, so every
unordered group-pair {A,B} lands on at least one core. Each core loads the
8 groups it covers (512 columns, 8.4 MB fp8 — vs 640 for the circulant
scheme) and computes the upper block-triangle of its local 8x8-group gram
with triangle-trimmed m-tiles. The host assigns each global group-pair to
exactly one computed unit with weight 2 (1 for diagonal groups) and sums.

Per-core device work: [16384, 512] fp8 panel SBUF-resident (tapered DMA
groups), fp8 DoubleRow matmuls into four PSUM tiles (widths 512/384/256/
128, one per m-tile), ACT Square + DVE chunked reduce per m-tile, DMA out
[128, 20] f32 per-partition per-64-column-chunk sums.
"""

import sys

import numpy as np
import ml_dtypes

if "/opt/trn_rl_repo" not in sys.path:
    sys.path.insert(0, "/opt/trn_rl_repo")

N = 1024
D = 16384
NC = 8
GRP = 64             # covering-design group size (gram rows)
NG = N // GRP        # 16 groups
LG = 8               # groups loaded per core
NCOLS = LG * GRP     # 512
BLK = 128
KT = 128             # contraction k-tiles of 128
KG = 4               # k-tiles per main DMA group
MT = 4               # m-tiles per core
MT_OFF = [0, 8, 14, 18]  # acc column offset per m-tile
ACC_W = 20

# --- Fano covering design (computed once, deterministic) -------------------
_LINES = [(0, 1, 2), (0, 3, 4), (0, 5, 6), (1, 3, 5), (1, 4, 6), (2, 3, 6), (2, 4, 5)]


def _design():
    T = {}
    for g in range(4):
        T[g] = {7, 0, 1, 2}
    for i, g in enumerate(range(4, NG)):
        T[g] = set(_LINES[1 + i // 2])
    S = {c: sorted(g for g in range(NG) if c in T[g]) for c in range(NC)}
    for c in range(NC):
        pads = [g for g in range(NG) if g not in S[c]]
        S[c] = sorted(S[c] + pads[: LG - len(S[c])])
    # assign every unordered global pair / diagonal to one computed unit
    need = {}
    for A in range(NG):
        for B in range(A, NG):
            need[(A, B)] = 2.0 if A != B else 1.0
    W = {c: {} for c in range(NC)}
    for c in range(NC):
        for t in range(MT):
            for j in range(2 * t, LG):
                for h in range(2):
                    a = 2 * t + h
                    GA, GB = S[c][a], S[c][j]
                    p = (min(GA, GB), max(GA, GB))
                    if need.get(p, 0) > 0:
                        W[c][(t, j, h)] = need[p]
                        need[p] = 0
    assert all(v == 0 for v in need.values())
    return S, W


_S, _W = _design()

_CACHE = {}


def _build_nc(repeats: int = 1, kg: int = KG, in_dt: str = "float8e4"):
    import concourse.bacc as bacc
    import concourse.mybir as mybir
    import concourse.tile as tile

    nc = bacc.Bacc("TRN2", target_bir_lowering=False, debug=False, num_devices=NC)

    xdt = getattr(mybir.dt, in_dt)
    f32 = mybir.dt.float32
    op = mybir.AluOpType

    xr = nc.dram_tensor("xr", [D, NCOLS], xdt, kind="ExternalInput")
    acc = nc.dram_tensor("acc", [BLK, ACC_W], f32, kind="ExternalOutput")

    # tapered DMA plan: big groups, then two 2-k-tile tail groups
    tail_sizes = [2, 2]
    nmain = (KT - sum(tail_sizes)) // kg
    sizes = [kg] * nmain + tail_sizes

    with tile.TileContext(nc) as tc:
        with (
            tc.tile_pool(name="xsA", bufs=nmain) as xsA,
            tc.tile_pool(name="xsB", bufs=len(tail_sizes)) as xsB,
            tc.tile_pool(name="ep", bufs=1) as ep,
            tc.tile_pool(name="ps", bufs=1, space="PSUM") as ps,
        ):
            acc_sb = ep.tile([BLK, ACC_W], f32, tag="acc")

            for _rep in range(repeats):
                xtiles = []
                kt0 = 0
                for i, sz in enumerate(sizes):
                    pool = xsA if i < nmain else xsB
                    t = pool.tile(
                        [BLK, sz, NCOLS], xdt,
                        tag="xpM" if i < nmain else f"xpT{i - nmain}",
                    )
                    nc.sync.dma_start(
                        out=t,
                        in_=xr[kt0 * BLK : (kt0 + sz) * BLK, :].rearrange(
                            "(k p) n -> p k n", p=BLK
                        ),
                    )
                    xtiles.append((sz, t))
                    kt0 += sz

                psums = []
                for t in range(MT):
                    pt = ps.tile([BLK, NCOLS - BLK * t], f32, tag=f"p{t}")
                    psums.append(pt)

                # fp8 DoubleRow, triangle-trimmed m-tiles: m-tile t covers
                # cols [128t : 512] so only the upper block-triangle is done
                npairs = KT // 2
                pi = 0
                for sz, t in xtiles:
                    for jp in range(sz // 2):
                        first = pi == 0
                        last = pi == npairs - 1
                        for mt in range(MT - 1, -1, -1):
                            nc.tensor.matmul(
                                psums[mt],
                                lhsT=t[:, 2 * jp : 2 * jp + 2, BLK * mt : BLK * (mt + 1)],
                                rhs=t[:, 2 * jp : 2 * jp + 2, BLK * mt : NCOLS],
                                perf_mode=mybir.MatmulPerfMode.DoubleRow,
                                start=first,
                                stop=last,
                            )
                        pi += 1

                # epilogue per m-tile: ACT exact Square, then DVE reduce per
                # 64-column chunk into acc[:, MT_OFF[t] : MT_OFF[t]+chunks]
                for mt in range(MT - 1, -1, -1):
                    width = NCOLS - BLK * mt
                    nch = width // GRP
                    scr = ep.tile([BLK, width], f32, tag=f"s{mt}")
                    nc.scalar.activation(
                        out=scr,
                        in_=psums[mt],
                        func=mybir.ActivationFunctionType.Square,
                    )
                    nc.vector.tensor_reduce(
                        out=acc_sb[:, MT_OFF[mt] : MT_OFF[mt] + nch],
                        in_=scr.rearrange("p (c e) -> p c e", e=GRP),
                        axis=mybir.AxisListType.X,
                        op=op.add,
                    )

            nc.sync.dma_start(out=acc.ap(), in_=acc_sb)

    nc.finalize()
    return nc


def _get_nc(repeats: int = 1):
    key = ("nc", repeats)
    if key not in _CACHE:
        _CACHE[key] = _build_nc(repeats)
    return _CACHE[key]


def _prepare_in_maps(fm_t: np.ndarray):
    fp8 = ml_dtypes.float8_e4m3
    x = np.ascontiguousarray(np.asarray(fm_t).reshape(N, D))
    xT = np.ascontiguousarray(x.astype(fp8).T)  # [D, N] fp8

    in_maps = []
    for c in range(NC):
        cols = np.concatenate(
            [np.arange(GRP * g, GRP * (g + 1)) for g in _S[c]]
        )
        in_maps.append({"xr": np.ascontiguousarray(xT[:, cols])})
    return in_maps


def _host_terms(fm_t: np.ndarray):
    """f64 O(N*D) side terms of the expansion from the exact f32 input."""
    x = np.asarray(fm_t).reshape(N, D).astype(np.float64)
    sq = (x * x).sum(axis=1)
    s = x.sum(axis=0)
    r = x @ s
    A = float((sq * sq).sum())
    S1 = float(sq.sum())
    B = float((sq * r).sum())
    return A, S1, B


def run(fm_t: np.ndarray, trace: bool = False, repeats: int = 1, in_maps=None):
    """Returns (loss_f32, BassKernelResults)."""
    from concourse.bass_utils import run_bass_kernel_spmd

    nc = _get_nc(repeats)
    if in_maps is None:
        in_maps = _prepare_in_maps(fm_t)
    res = run_bass_kernel_spmd(nc, in_maps, list(range(NC)), trace=trace)

    # Qg = weighted sum of per-unit g^2 sums over the covering design
    qg = 0.0
    for c in range(NC):
        a = res.results[c]["acc"].astype(np.float64)
        for (t, j, h), w in _W[c].items():
            col = MT_OFF[t] + (j - 2 * t)
            qg += w * float(a[64 * h : 64 * (h + 1), col].sum())

    A, S1, B = _host_terms(fm_t)
    tot = 2.0 * N * A + 2.0 * S1 * S1 - 8.0 * B + 4.0 * qg
    loss = tot / (float(N) ** 2 * float(D) ** 2)
    return np.float32(loss), res


def kernel(fm_s: np.ndarray, fm_t: np.ndarray) -> np.ndarray:
    loss, _ = run(fm_t, trace=False)
    return np.asarray(loss, dtype=np.float32)


# revision 23
# speedup vs baseline: 1102.7840x; 1.0843x over previous
"""Trainium2 kernel for nn_DAD_MA_35330400976941 (pairwise-MSE gram loss).

reference math (fm_s is ignored — the original source overwrites G_s with the
teacher matrix and compares against zeros):
    x   = fm_t.reshape(1024, 16384)
    g   = x @ x.T
    sq  = diag(g)
    out = mean(((sq[:,None] + sq[None,:] - 2*g) / D)**2)

Algorithm: expand the sum,
    sum_ij t^2 = 2*N*sum(sq^2) + 2*(sum sq)^2 - 8*sum_i sq_i*(x_i . s)
                 + 4*Qg,      Qg = sum_ij g_ij^2
with s = sum_j x_j. All O(N*D) terms are computed on the host in float64
from the exact f32 input; only Qg (0.1% of the total) runs on the device,
so the device gram uses fp8e4m3 inputs (fp32 PSUM accumulation).

Distribution (8 cores, SPMD): Fano-plane covering design over 32 groups of
32 gram rows. Each group's covering-core set is a Fano line over cores 0-6
(multiplicities 5,5,5,5,4,4 on lines L1..L6) or {7} u L0 (4 groups); any
two lines intersect, so every unordered group-pair lands on some core.
Each core loads the (padded) 14 groups it covers — 448 columns, 7.34 MB
fp8 — and computes the upper block-triangle of its local 14x14-group gram
with 4 triangle-trimmed m-tiles (the last one 64 rows wide). The host
assigns each global group-pair to exactly one computed 32x32 unit with
weight 2 (1 for diagonals) and sums; the fp8 gram is symmetric so the
choice of orientation is value-exact.

The host pre-tiles each core's panel in DMA-group order ([p, k, n] within
each group) so every DMA descriptor covers >=1792 contiguous bytes
(448-byte rows alone would fall under the 512B read-modify-write penalty).

Per-core device work: [16384, 448] fp8 panel SBUF-resident (tapered DMA
groups), fp8 DoubleRow matmuls into four PSUM tiles (widths 448/320/192/
64), ACT Square + DVE per-32-chunk reduce, DMA out [128, 32] f32.
"""

import sys

import numpy as np
import ml_dtypes

if "/opt/trn_rl_repo" not in sys.path:
    sys.path.insert(0, "/opt/trn_rl_repo")

N = 1024
D = 16384
NC = 8
GRP = 32             # covering-design group size (gram rows)
NG = N // GRP        # 32 groups
LG = 14              # groups loaded per core
NCOLS = LG * GRP     # 448
BLK = 128
KT = 128             # contraction k-tiles of 128
KG = 4               # k-tiles per main DMA group
MT = 4               # m-tiles per core
MT_START = [0, 4, 8, 12]   # m-tile start, in 32-col groups
MT_ROWS = [128, 128, 128, 64]
MT_OFF = [0, 14, 24, 30]   # acc column offset per m-tile
ACC_W = 32

# --- Fano covering design (deterministic) ----------------------------------
_LINES = [(0, 1, 2), (0, 3, 4), (0, 5, 6), (1, 3, 5), (1, 4, 6), (2, 3, 6), (2, 4, 5)]
_MULT = {1: 5, 2: 5, 3: 5, 4: 5, 5: 4, 6: 4}


def _design():
    T = {}
    g = 0
    for _ in range(4):
        T[g] = {7, 0, 1, 2}
        g += 1
    for li, m in _MULT.items():
        for _ in range(m):
            T[g] = set(_LINES[li])
            g += 1
    S = {c: sorted(x for x in range(NG) if c in T[x]) for c in range(NC)}
    for c in range(NC):
        pads = [x for x in range(NG) if x not in S[c]]
        S[c] = sorted(S[c] + pads[: LG - len(S[c])])
    need = {}
    for A in range(NG):
        for B in range(A, NG):
            need[(A, B)] = 2.0 if A != B else 1.0
    W = {c: {} for c in range(NC)}
    for c in range(NC):
        for t in range(MT):
            for j in range(MT_START[t], LG):
                for h in range(MT_ROWS[t] // GRP):
                    a = MT_START[t] + h
                    GA, GB = S[c][a], S[c][j]
                    p = (min(GA, GB), max(GA, GB))
                    if need.get(p, 0) > 0:
                        W[c][(t, j, h)] = need[p]
                        need[p] = 0
    assert all(v == 0 for v in need.values())
    return S, W


_S, _W = _design()

_CACHE = {}


def _build_nc(repeats: int = 1, kg: int = KG, in_dt: str = "float8e4"):
    import concourse.bacc as bacc
    import concourse.mybir as mybir
    import concourse.tile as tile

    nc = bacc.Bacc("TRN2", target_bir_lowering=False, debug=False, num_devices=NC)

    xdt = getattr(mybir.dt, in_dt)
    f32 = mybir.dt.float32
    op = mybir.AluOpType

    xr = nc.dram_tensor("xr", [D, NCOLS], xdt, kind="ExternalInput")
    acc = nc.dram_tensor("acc", [BLK, ACC_W], f32, kind="ExternalOutput")

    # tapered DMA plan: big groups, then two 2-k-tile tail groups
    tail_sizes = [2, 2]
    nmain = (KT - sum(tail_sizes)) // kg
    sizes = [kg] * nmain + tail_sizes

    with tile.TileContext(nc) as tc:
        with (
            tc.tile_pool(name="xsA", bufs=nmain) as xsA,
            tc.tile_pool(name="xsB", bufs=len(tail_sizes)) as xsB,
            tc.tile_pool(name="ep", bufs=1) as ep,
            tc.tile_pool(name="ps", bufs=1, space="PSUM") as ps,
        ):
            acc_sb = ep.tile([BLK, ACC_W], f32, tag="acc")

            for _rep in range(repeats):
                xtiles = []
                kt0 = 0
                for i, sz in enumerate(sizes):
                    pool = xsA if i < nmain else xsB
                    t = pool.tile(
                        [BLK, sz, NCOLS], xdt,
                        tag="xpM" if i < nmain else f"xpT{i - nmain}",
                    )
                    # host stores this group's rows pre-tiled as [p, k, n],
                    # so each partition reads sz*NCOLS contiguous bytes
                    nc.sync.dma_start(
                        out=t,
                        in_=xr[kt0 * BLK : (kt0 + sz) * BLK, :].rearrange(
                            "(p k) n -> p k n", k=sz
                        ),
                    )
                    xtiles.append((sz, t))
                    kt0 += sz

                psums = []
                for t in range(MT):
                    c0 = MT_START[t] * GRP
                    pt = ps.tile([MT_ROWS[t], NCOLS - c0], f32, tag=f"p{t}")
                    psums.append(pt)

                # fp8 DoubleRow, triangle-trimmed m-tiles
                npairs = KT // 2
                pi = 0
                for sz, t in xtiles:
                    for jp in range(sz // 2):
                        first = pi == 0
                        last = pi == npairs - 1
                        for mt in range(MT - 1, -1, -1):
                            c0 = MT_START[mt] * GRP
                            nc.tensor.matmul(
                                psums[mt],
                                lhsT=t[:, 2 * jp : 2 * jp + 2, c0 : c0 + MT_ROWS[mt]],
                                rhs=t[:, 2 * jp : 2 * jp + 2, c0:NCOLS],
                                perf_mode=mybir.MatmulPerfMode.DoubleRow,
                                start=first,
                                stop=last,
                            )
                        pi += 1

                # epilogue per m-tile: ACT exact Square, then DVE reduce per
                # 32-column chunk into acc[:, MT_OFF[t] : MT_OFF[t]+nch]
                for mt in range(MT - 1, -1, -1):
                    c0 = MT_START[mt] * GRP
                    width = NCOLS - c0
                    nch = width // GRP
                    rows = MT_ROWS[mt]
                    scr = ep.tile([rows, width], f32, tag=f"s{mt}")
                    nc.scalar.activation(
                        out=scr,
                        in_=psums[mt],
                        func=mybir.ActivationFunctionType.Square,
                    )
                    nc.vector.tensor_reduce(
                        out=acc_sb[0:rows, MT_OFF[mt] : MT_OFF[mt] + nch],
                        in_=scr.rearrange("p (c e) -> p c e", e=GRP),
                        axis=mybir.AxisListType.X,
                        op=op.add,
                    )

            nc.sync.dma_start(out=acc.ap(), in_=acc_sb)

    nc.finalize()
    return nc


def _get_nc(repeats: int = 1):
    key = ("nc", repeats)
    if key not in _CACHE:
        _CACHE[key] = _build_nc(repeats)
    return _CACHE[key]


_DMA_SIZES = [KG] * ((KT - 4) // KG) + [2, 2]


def _prepare_in_maps(fm_t: np.ndarray):
    fp8 = ml_dtypes.float8_e4m3
    x = np.ascontiguousarray(np.asarray(fm_t).reshape(N, D))
    xT = np.ascontiguousarray(x.astype(fp8).T)  # [D, N] fp8

    in_maps = []
    for c in range(NC):
        cols = np.concatenate(
            [np.arange(GRP * g, GRP * (g + 1)) for g in _S[c]]
        )
        sel = xT[:, cols]  # [D, NCOLS]
        xr_c = np.empty((D, NCOLS), dtype=fp8)
        kt0 = 0
        for sz in _DMA_SIZES:
            blk = sel[kt0 * BLK : (kt0 + sz) * BLK]
            xr_c[kt0 * BLK : (kt0 + sz) * BLK] = (
                blk.reshape(sz, BLK, NCOLS).transpose(1, 0, 2).reshape(sz * BLK, NCOLS)
            )
            kt0 += sz
        in_maps.append({"xr": xr_c})
    return in_maps


def _host_terms(fm_t: np.ndarray):
    """f64 O(N*D) side terms of the expansion from the exact f32 input."""
    x = np.asarray(fm_t).reshape(N, D).astype(np.float64)
    sq = (x * x).sum(axis=1)
    s = x.sum(axis=0)
    r = x @ s
    A = float((sq * sq).sum())
    S1 = float(sq.sum())
    B = float((sq * r).sum())
    return A, S1, B


def run(fm_t: np.ndarray, trace: bool = False, repeats: int = 1, in_maps=None):
    """Returns (loss_f32, BassKernelResults)."""
    from concourse.bass_utils import run_bass_kernel_spmd

    nc = _get_nc(repeats)
    if in_maps is None:
        in_maps = _prepare_in_maps(fm_t)
    res = run_bass_kernel_spmd(nc, in_maps, list(range(NC)), trace=trace)

    # Qg = weighted sum of per-unit g^2 sums over the covering design
    qg = 0.0
    for c in range(NC):
        a = res.results[c]["acc"].astype(np.float64)
        for (t, j, h), w in _W[c].items():
            col = MT_OFF[t] + (j - MT_START[t])
            qg += w * float(a[GRP * h : GRP * (h + 1), col].sum())

    A, S1, B = _host_terms(fm_t)
    tot = 2.0 * N * A + 2.0 * S1 * S1 - 8.0 * B + 4.0 * qg
    loss = tot / (float(N) ** 2 * float(D) ** 2)
    return np.float32(loss), res


def kernel(fm_s: np.ndarray, fm_t: np.ndarray) -> np.ndarray:
    loss, _ = run(fm_t, trace=False)
    return np.asarray(loss, dtype=np.float32)


# revision 25
# speedup vs baseline: 1120.4927x; 1.0161x over previous
"""Trainium2 kernel for nn_DAD_MA_35330400976941 (pairwise-MSE gram loss).

reference math (fm_s is ignored — the original source overwrites G_s with the
teacher matrix and compares against zeros):
    x   = fm_t.reshape(1024, 16384)
    g   = x @ x.T
    sq  = diag(g)
    out = mean(((sq[:,None] + sq[None,:] - 2*g) / D)**2)

Algorithm: expand the sum,
    sum_ij t^2 = 2*N*sum(sq^2) + 2*(sum sq)^2 - 8*sum_i sq_i*(x_i . s)
                 + 4*Qg,      Qg = sum_ij g_ij^2
with s = sum_j x_j. All O(N*D) terms are computed on the host in float64
from the exact f32 input; only Qg (0.1% of the total) runs on the device,
so the device gram uses fp8e4m3 inputs (fp32 PSUM accumulation).

Distribution (8 cores, SPMD): Fano-plane covering design over 32 groups of
32 gram rows. Each group's covering-core set is a Fano line over cores 0-6
(multiplicities 5,5,5,5,4,4 on lines L1..L6) or {7} u L0 (4 groups); any
two lines intersect, so every unordered group-pair lands on some core.
Each core loads the (padded) 14 groups it covers — 448 columns, 7.34 MB
fp8 — and computes the upper block-triangle of its local 14x14-group gram
with 4 triangle-trimmed m-tiles (the last one 64 rows wide). The host
assigns each global group-pair to exactly one computed 32x32 unit with
weight 2 (1 for diagonals) and sums; the fp8 gram is symmetric so the
choice of orientation is value-exact.

The host pre-tiles each core's panel in DMA-group order ([p, k, n] within
each group) so every DMA descriptor covers >=1792 contiguous bytes
(448-byte rows alone would fall under the 512B read-modify-write penalty).

Per-core device work: [16384, 448] fp8 panel SBUF-resident (tapered DMA
groups), fp8 DoubleRow matmuls into four PSUM tiles (widths 448/320/192/
64), ACT Square + DVE per-32-chunk reduce, DMA out [128, 32] f32.
"""

import sys

import numpy as np
import ml_dtypes

if "/opt/trn_rl_repo" not in sys.path:
    sys.path.insert(0, "/opt/trn_rl_repo")

N = 1024
D = 16384
NC = 8
GRP = 32             # covering-design group size (gram rows)
NG = N // GRP        # 32 groups
LG = 14              # groups loaded per core
NCOLS = LG * GRP     # 448
BLK = 128
KT = 128             # contraction k-tiles of 128
KG = 8               # k-tiles per main DMA group
MT = 4               # m-tiles per core
MT_START = [0, 4, 8, 12]   # m-tile start, in 32-col groups
MT_ROWS = [128, 128, 128, 64]
MT_OFF = [0, 14, 24, 30]   # acc column offset per m-tile
ACC_W = 32

# --- Fano covering design (deterministic) ----------------------------------
_LINES = [(0, 1, 2), (0, 3, 4), (0, 5, 6), (1, 3, 5), (1, 4, 6), (2, 3, 6), (2, 4, 5)]
_MULT = {1: 5, 2: 5, 3: 5, 4: 5, 5: 4, 6: 4}


def _design():
    T = {}
    g = 0
    for _ in range(4):
        T[g] = {7, 0, 1, 2}
        g += 1
    for li, m in _MULT.items():
        for _ in range(m):
            T[g] = set(_LINES[li])
            g += 1
    S = {c: sorted(x for x in range(NG) if c in T[x]) for c in range(NC)}
    for c in range(NC):
        pads = [x for x in range(NG) if x not in S[c]]
        S[c] = sorted(S[c] + pads[: LG - len(S[c])])
    need = {}
    for A in range(NG):
        for B in range(A, NG):
            need[(A, B)] = 2.0 if A != B else 1.0
    W = {c: {} for c in range(NC)}
    for c in range(NC):
        for t in range(MT):
            for j in range(MT_START[t], LG):
                for h in range(MT_ROWS[t] // GRP):
                    a = MT_START[t] + h
                    GA, GB = S[c][a], S[c][j]
                    p = (min(GA, GB), max(GA, GB))
                    if need.get(p, 0) > 0:
                        W[c][(t, j, h)] = need[p]
                        need[p] = 0
    assert all(v == 0 for v in need.values())
    return S, W


_S, _W = _design()

_CACHE = {}


def _build_nc(repeats: int = 1, kg: int = KG, in_dt: str = "float8e4"):
    import concourse.bacc as bacc
    import concourse.mybir as mybir
    import concourse.tile as tile

    nc = bacc.Bacc("TRN2", target_bir_lowering=False, debug=False, num_devices=NC)

    xdt = getattr(mybir.dt, in_dt)
    f32 = mybir.dt.float32
    op = mybir.AluOpType

    xr = nc.dram_tensor("xr", [D, NCOLS], xdt, kind="ExternalInput")
    acc = nc.dram_tensor("acc", [BLK, ACC_W], f32, kind="ExternalOutput")

    # tapered DMA plan: big groups, then two 2-k-tile tail groups
    tail_sizes = [4, 2, 2]
    nmain = (KT - sum(tail_sizes)) // kg
    sizes = [kg] * nmain + tail_sizes
    assert sum(sizes) == KT, sizes

    with tile.TileContext(nc) as tc:
        with (
            tc.tile_pool(name="xsA", bufs=nmain) as xsA,
            tc.tile_pool(name="xsB", bufs=len(tail_sizes)) as xsB,
            tc.tile_pool(name="ep", bufs=1) as ep,
            tc.tile_pool(name="ps", bufs=1, space="PSUM") as ps,
        ):
            acc_sb = ep.tile([BLK, ACC_W], f32, tag="acc")

            for _rep in range(repeats):
                xtiles = []
                kt0 = 0
                for i, sz in enumerate(sizes):
                    pool = xsA if i < nmain else xsB
                    t = pool.tile(
                        [BLK, sz, NCOLS], xdt,
                        tag="xpM" if i < nmain else f"xpT{i - nmain}",
                    )
                    # host stores this group's rows pre-tiled as [p, k, n],
                    # so each partition reads sz*NCOLS contiguous bytes
                    nc.sync.dma_start(
                        out=t,
                        in_=xr[kt0 * BLK : (kt0 + sz) * BLK, :].rearrange(
                            "(p k) n -> p k n", k=sz
                        ),
                    )
                    xtiles.append((sz, t))
                    kt0 += sz

                psums = []
                for t in range(MT):
                    c0 = MT_START[t] * GRP
                    pt = ps.tile([MT_ROWS[t], NCOLS - c0], f32, tag=f"p{t}")
                    psums.append(pt)

                # fp8 DoubleRow, triangle-trimmed m-tiles
                npairs = KT // 2
                pi = 0
                for sz, t in xtiles:
                    for jp in range(sz // 2):
                        first = pi == 0
                        last = pi == npairs - 1
                        for mt in range(MT - 1, -1, -1):
                            c0 = MT_START[mt] * GRP
                            nc.tensor.matmul(
                                psums[mt],
                                lhsT=t[:, 2 * jp : 2 * jp + 2, c0 : c0 + MT_ROWS[mt]],
                                rhs=t[:, 2 * jp : 2 * jp + 2, c0:NCOLS],
                                perf_mode=mybir.MatmulPerfMode.DoubleRow,
                                start=first,
                                stop=last,
                            )
                        pi += 1

                # epilogue per m-tile: ACT exact Square, then DVE reduce per
                # 32-column chunk into acc[:, MT_OFF[t] : MT_OFF[t]+nch]
                for mt in range(MT - 1, -1, -1):
                    c0 = MT_START[mt] * GRP
                    width = NCOLS - c0
                    nch = width // GRP
                    rows = MT_ROWS[mt]
                    scr = ep.tile([rows, width], f32, tag=f"s{mt}")
                    nc.scalar.activation(
                        out=scr,
                        in_=psums[mt],
                        func=mybir.ActivationFunctionType.Square,
                    )
                    nc.vector.tensor_reduce(
                        out=acc_sb[0:rows, MT_OFF[mt] : MT_OFF[mt] + nch],
                        in_=scr.rearrange("p (c e) -> p c e", e=GRP),
                        axis=mybir.AxisListType.X,
                        op=op.add,
                    )

            nc.sync.dma_start(out=acc.ap(), in_=acc_sb)

    nc.finalize()
    return nc


def _get_nc(repeats: int = 1):
    key = ("nc", repeats)
    if key not in _CACHE:
        _CACHE[key] = _build_nc(repeats)
    return _CACHE[key]


_DMA_SIZES = [KG] * ((KT - 8) // KG) + [4, 2, 2]
assert sum(_DMA_SIZES) == KT


def _prepare_in_maps(fm_t: np.ndarray):
    fp8 = ml_dtypes.float8_e4m3
    x = np.ascontiguousarray(np.asarray(fm_t).reshape(N, D))
    xT = np.ascontiguousarray(x.astype(fp8).T)  # [D, N] fp8

    in_maps = []
    for c in range(NC):
        cols = np.concatenate(
            [np.arange(GRP * g, GRP * (g + 1)) for g in _S[c]]
        )
        sel = xT[:, cols]  # [D, NCOLS]
        xr_c = np.empty((D, NCOLS), dtype=fp8)
        kt0 = 0
        for sz in _DMA_SIZES:
            blk = sel[kt0 * BLK : (kt0 + sz) * BLK]
            xr_c[kt0 * BLK : (kt0 + sz) * BLK] = (
                blk.reshape(sz, BLK, NCOLS).transpose(1, 0, 2).reshape(sz * BLK, NCOLS)
            )
            kt0 += sz
        in_maps.append({"xr": xr_c})
    return in_maps


def _host_terms(fm_t: np.ndarray):
    """f64 O(N*D) side terms of the expansion from the exact f32 input."""
    x = np.asarray(fm_t).reshape(N, D).astype(np.float64)
    sq = (x * x).sum(axis=1)
    s = x.sum(axis=0)
    r = x @ s
    A = float((sq * sq).sum())
    S1 = float(sq.sum())
    B = float((sq * r).sum())
    return A, S1, B


def run(fm_t: np.ndarray, trace: bool = False, repeats: int = 1, in_maps=None):
    """Returns (loss_f32, BassKernelResults)."""
    from concourse.bass_utils import run_bass_kernel_spmd

    nc = _get_nc(repeats)
    if in_maps is None:
        in_maps = _prepare_in_maps(fm_t)
    res = run_bass_kernel_spmd(nc, in_maps, list(range(NC)), trace=trace)

    # Qg = weighted sum of per-unit g^2 sums over the covering design
    qg = 0.0
    for c in range(NC):
        a = res.results[c]["acc"].astype(np.float64)
        for (t, j, h), w in _W[c].items():
            col = MT_OFF[t] + (j - MT_START[t])
            qg += w * float(a[GRP * h : GRP * (h + 1), col].sum())

    A, S1, B = _host_terms(fm_t)
    tot = 2.0 * N * A + 2.0 * S1 * S1 - 8.0 * B + 4.0 * qg
    loss = tot / (float(N) ** 2 * float(D) ** 2)
    return np.float32(loss), res


def kernel(fm_s: np.ndarray, fm_t: np.ndarray) -> np.ndarray:
    loss, _ = run(fm_t, trace=False)
    return np.asarray(loss, dtype=np.float32)


# revision 26
# speedup vs baseline: 1130.3441x; 1.0088x over previous
"""Trainium2 kernel for nn_DAD_MA_35330400976941 (pairwise-MSE gram loss).

reference math (fm_s is ignored — the original source overwrites G_s with the
teacher matrix and compares against zeros):
    x   = fm_t.reshape(1024, 16384)
    g   = x @ x.T
    sq  = diag(g)
    out = mean(((sq[:,None] + sq[None,:] - 2*g) / D)**2)

Algorithm: expand the sum,
    sum_ij t^2 = 2*N*sum(sq^2) + 2*(sum sq)^2 - 8*sum_i sq_i*(x_i . s)
                 + 4*Qg,      Qg = sum_ij g_ij^2
with s = sum_j x_j. All O(N*D) terms are computed on the host in float64
from the exact f32 input; only Qg (0.1% of the total) runs on the device,
so the device gram uses fp8e4m3 inputs (fp32 PSUM accumulation).

Distribution (8 cores, SPMD): Fano-plane covering design over 32 groups of
32 gram rows. Each group's covering-core set is a Fano line over cores 0-6
(multiplicities 5,5,5,5,4,4 on lines L1..L6) or {7} u L0 (4 groups); any
two lines intersect, so every unordered group-pair lands on some core.
Each core loads the (padded) 14 groups it covers — 448 columns, 7.34 MB
fp8 — and computes the upper block-triangle of its local 14x14-group gram
with 4 triangle-trimmed m-tiles (the last one 64 rows wide). The host
assigns each global group-pair to exactly one computed 32x32 unit with
weight 2 (1 for diagonals) and sums; the fp8 gram is symmetric so the
choice of orientation is value-exact.

The host pre-tiles each core's panel in DMA-group order ([p, k, n] within
each group) so every DMA descriptor covers >=1792 contiguous bytes
(448-byte rows alone would fall under the 512B read-modify-write penalty).

Per-core device work: [16384, 448] fp8 panel SBUF-resident (tapered DMA
groups), fp8 DoubleRow matmuls into four PSUM tiles (widths 448/320/192/
64), ACT Square + DVE per-32-chunk reduce, DMA out [128, 32] f32.
"""

import sys

import numpy as np
import ml_dtypes

if "/opt/trn_rl_repo" not in sys.path:
    sys.path.insert(0, "/opt/trn_rl_repo")

N = 1024
D = 16384
NC = 8
GRP = 32             # covering-design group size (gram rows)
NG = N // GRP        # 32 groups
LG = 14              # groups loaded per core
NCOLS = LG * GRP     # 448
BLK = 128
KT = 128             # contraction k-tiles of 128
KG = 8               # k-tiles per main DMA group
MT = 4               # m-tiles per core
MT_START = [0, 4, 8, 12]   # m-tile start, in 32-col groups
MT_ROWS = [128, 128, 128, 64]
MT_OFF = [0, 14, 24, 30]   # acc column offset per m-tile
ACC_W = 32

# --- Fano covering design (deterministic) ----------------------------------
_LINES = [(0, 1, 2), (0, 3, 4), (0, 5, 6), (1, 3, 5), (1, 4, 6), (2, 3, 6), (2, 4, 5)]
_MULT = {1: 5, 2: 5, 3: 5, 4: 5, 5: 4, 6: 4}


def _design():
    T = {}
    g = 0
    for _ in range(4):
        T[g] = {7, 0, 1, 2}
        g += 1
    for li, m in _MULT.items():
        for _ in range(m):
            T[g] = set(_LINES[li])
            g += 1
    S = {c: sorted(x for x in range(NG) if c in T[x]) for c in range(NC)}
    for c in range(NC):
        pads = [x for x in range(NG) if x not in S[c]]
        S[c] = sorted(S[c] + pads[: LG - len(S[c])])
    need = {}
    for A in range(NG):
        for B in range(A, NG):
            need[(A, B)] = 2.0 if A != B else 1.0
    W = {c: {} for c in range(NC)}
    for c in range(NC):
        for t in range(MT):
            for j in range(MT_START[t], LG):
                for h in range(MT_ROWS[t] // GRP):
                    a = MT_START[t] + h
                    GA, GB = S[c][a], S[c][j]
                    p = (min(GA, GB), max(GA, GB))
                    if need.get(p, 0) > 0:
                        W[c][(t, j, h)] = need[p]
                        need[p] = 0
    assert all(v == 0 for v in need.values())
    return S, W


_S, _W = _design()

_CACHE = {}


def _build_nc(repeats: int = 1, kg: int = KG, in_dt: str = "float8e4"):
    import concourse.bacc as bacc
    import concourse.mybir as mybir
    import concourse.tile as tile

    nc = bacc.Bacc("TRN2", target_bir_lowering=False, debug=False, num_devices=NC)

    xdt = getattr(mybir.dt, in_dt)
    f32 = mybir.dt.float32
    op = mybir.AluOpType

    xr = nc.dram_tensor("xr", [D, NCOLS], xdt, kind="ExternalInput")
    acc = nc.dram_tensor("acc", [BLK, ACC_W], f32, kind="ExternalOutput")

    # tapered DMA plan: big groups, then two 2-k-tile tail groups
    tail_sizes = [4, 2, 2]
    nmain = (KT - sum(tail_sizes)) // kg
    sizes = [kg] * nmain + tail_sizes
    assert sum(sizes) == KT, sizes

    with tile.TileContext(nc) as tc:
        with (
            tc.tile_pool(name="xsA", bufs=nmain) as xsA,
            tc.tile_pool(name="xsB", bufs=len(tail_sizes)) as xsB,
            tc.tile_pool(name="ep", bufs=1) as ep,
            tc.tile_pool(name="ps", bufs=1, space="PSUM") as ps,
        ):
            acc_sb = ep.tile([BLK, ACC_W], f32, tag="acc")

            for _rep in range(repeats):
                xtiles = []
                kt0 = 0
                for i, sz in enumerate(sizes):
                    pool = xsA if i < nmain else xsB
                    t = pool.tile(
                        [BLK, sz, NCOLS], xdt,
                        tag="xpM" if i < nmain else f"xpT{i - nmain}",
                    )
                    # host stores this group's rows pre-tiled as [p, k, n],
                    # so each partition reads sz*NCOLS contiguous bytes
                    nc.sync.dma_start(
                        out=t,
                        in_=xr[kt0 * BLK : (kt0 + sz) * BLK, :].rearrange(
                            "(p k) n -> p k n", k=sz
                        ),
                    )
                    xtiles.append((sz, t))
                    kt0 += sz

                psums = []
                for t in range(MT):
                    c0 = MT_START[t] * GRP
                    pt = ps.tile([MT_ROWS[t], NCOLS - c0], f32, tag=f"p{t}")
                    psums.append(pt)

                # fp8 DoubleRow, triangle-trimmed m-tiles
                npairs = KT // 2
                pi = 0
                for sz, t in xtiles:
                    for jp in range(sz // 2):
                        first = pi == 0
                        last = pi == npairs - 1
                        for mt in range(MT):
                            c0 = MT_START[mt] * GRP
                            nc.tensor.matmul(
                                psums[mt],
                                lhsT=t[:, 2 * jp : 2 * jp + 2, c0 : c0 + MT_ROWS[mt]],
                                rhs=t[:, 2 * jp : 2 * jp + 2, c0:NCOLS],
                                perf_mode=mybir.MatmulPerfMode.DoubleRow,
                                start=first,
                                stop=last,
                            )
                        pi += 1

                # epilogue per m-tile: ACT exact Square, then DVE reduce per
                # 32-column chunk into acc[:, MT_OFF[t] : MT_OFF[t]+nch]
                for mt in range(MT):
                    c0 = MT_START[mt] * GRP
                    width = NCOLS - c0
                    nch = width // GRP
                    rows = MT_ROWS[mt]
                    scr = ep.tile([rows, width], f32, tag=f"s{mt}")
                    nc.scalar.activation(
                        out=scr,
                        in_=psums[mt],
                        func=mybir.ActivationFunctionType.Square,
                    )
                    nc.vector.tensor_reduce(
                        out=acc_sb[0:rows, MT_OFF[mt] : MT_OFF[mt] + nch],
                        in_=scr.rearrange("p (c e) -> p c e", e=GRP),
                        axis=mybir.AxisListType.X,
                        op=op.add,
                    )

            nc.sync.dma_start(out=acc.ap(), in_=acc_sb)

    nc.finalize()
    return nc


def _get_nc(repeats: int = 1):
    key = ("nc", repeats)
    if key not in _CACHE:
        _CACHE[key] = _build_nc(repeats)
    return _CACHE[key]


_DMA_SIZES = [KG] * ((KT - 8) // KG) + [4, 2, 2]
assert sum(_DMA_SIZES) == KT


def _prepare_in_maps(fm_t: np.ndarray):
    fp8 = ml_dtypes.float8_e4m3
    x = np.ascontiguousarray(np.asarray(fm_t).reshape(N, D))
    xT = np.ascontiguousarray(x.astype(fp8).T)  # [D, N] fp8

    in_maps = []
    for c in range(NC):
        cols = np.concatenate(
            [np.arange(GRP * g, GRP * (g + 1)) for g in _S[c]]
        )
        sel = xT[:, cols]  # [D, NCOLS]
        xr_c = np.empty((D, NCOLS), dtype=fp8)
        kt0 = 0
        for sz in _DMA_SIZES:
            blk = sel[kt0 * BLK : (kt0 + sz) * BLK]
            xr_c[kt0 * BLK : (kt0 + sz) * BLK] = (
                blk.reshape(sz, BLK, NCOLS).transpose(1, 0, 2).reshape(sz * BLK, NCOLS)
            )
            kt0 += sz
        in_maps.append({"xr": xr_c})
    return in_maps


def _host_terms(fm_t: np.ndarray):
    """f64 O(N*D) side terms of the expansion from the exact f32 input."""
    x = np.asarray(fm_t).reshape(N, D).astype(np.float64)
    sq = (x * x).sum(axis=1)
    s = x.sum(axis=0)
    r = x @ s
    A = float((sq * sq).sum())
    S1 = float(sq.sum())
    B = float((sq * r).sum())
    return A, S1, B


def run(fm_t: np.ndarray, trace: bool = False, repeats: int = 1, in_maps=None):
    """Returns (loss_f32, BassKernelResults)."""
    from concourse.bass_utils import run_bass_kernel_spmd

    nc = _get_nc(repeats)
    if in_maps is None:
        in_maps = _prepare_in_maps(fm_t)
    res = run_bass_kernel_spmd(nc, in_maps, list(range(NC)), trace=trace)

    # Qg = weighted sum of per-unit g^2 sums over the covering design
    qg = 0.0
    for c in range(NC):
        a = res.results[c]["acc"].astype(np.float64)
        for (t, j, h), w in _W[c].items():
            col = MT_OFF[t] + (j - MT_START[t])
            qg += w * float(a[GRP * h : GRP * (h + 1), col].sum())

    A, S1, B = _host_terms(fm_t)
    tot = 2.0 * N * A + 2.0 * S1 * S1 - 8.0 * B + 4.0 * qg
    loss = tot / (float(N) ** 2 * float(D) ** 2)
    return np.float32(loss), res


def kernel(fm_s: np.ndarray, fm_t: np.ndarray) -> np.ndarray:
    loss, _ = run(fm_t, trace=False)
    return np.asarray(loss, dtype=np.float32)
